# revision 13
# baseline (speedup 1.0000x reference)
"""Block-sparse top-k masked linear for Trainium2, tensor-parallel over 8 cores.

out = (block_masked x) @ W + bias
  x: (128, 1, 4096) fp16, W: (4096, 11008) fp16, bias: (11008,) fp16
  mask: per (32-row x 64-col) block of x, keep blocks whose mean |x| is
  >= the 32nd-largest of the 64 k-block activations in that row block.

Sharding: column-parallel — each of the 8 cores gets an 11008/8 = 1376
column slice of W and bias; x is replicated; outputs are concatenated.
"""
from contextlib import ExitStack

import numpy as np

import concourse.bass as bass
import concourse.tile as tile
from concourse import bacc, mybir
from concourse.bass_utils import run_bass_kernel_spmd

F16 = mybir.dt.float16
F32 = mybir.dt.float32
AX = mybir.AxisListType
ALU = mybir.AluOpType
ACT = mybir.ActivationFunctionType

M = 128          # rows of x
K = 4096         # contraction
N = 11008        # out features
NCORES = 8
NLOC = N // NCORES           # 1376 columns per core
BLOCK_M, BLOCK_K = 32, 64
NBM, NBK = M // BLOCK_M, K // BLOCK_K   # 4 row blocks, 64 k blocks
KEEP = 32                               # k blocks kept per row block
NKT = K // 128                          # 32 k tiles of 128
N_TILES = [(0, 512), (512, 512), (1024, 352)]   # n-tile offsets/sizes
W_BUFS = 12


def _program(ctx: ExitStack, tc: tile.TileContext, ins, outs):
    nc = tc.nc
    x_d, w_d, b_d, e_d, id_d, jh_d, ksel_d = ins
    (o_d,) = outs

    const = ctx.enter_context(tc.tile_pool(name="const", bufs=1))
    xbuf = ctx.enter_context(tc.tile_pool(name="xbuf", bufs=1))
    mk = ctx.enter_context(tc.tile_pool(name="mk", bufs=1))
    wpool = ctx.enter_context(tc.tile_pool(name="wpool", bufs=W_BUFS))
    opool = ctx.enter_context(tc.tile_pool(name="opool", bufs=1))
    psum = ctx.enter_context(tc.tile_pool(name="psum", bufs=1, space="PSUM"))

    # ---- x transposed for the GEMM: xt[p, kt*128 + m] = x[m, 128 kt + p] ----
    # one xbar-transpose call on the scalar ring: out[p, kt, m] = in[m, kt*128+p]
    xt = xbuf.tile([128, K], F16)
    nc.scalar.dma_start_transpose(
        xt[:].rearrange("p (kt m) -> p kt m", kt=NKT), x_d)

    # ---- mask pipeline (natural x layout; independent of the transpose) ----
    # x chunks in separate tiles so loads don't WAR-serialize behind reduces
    NCH = 8
    xc = ctx.enter_context(tc.tile_pool(name="xc", bufs=NCH))
    part_n = mk.tile([128, NBK], F32)
    jc = NBK // NCH
    ks = K // NCH
    for c in range(NCH):
        x_c = xc.tile([128, ks], F16, name=f"xch{c}", tag="xch")
        nc.sync.dma_start(x_c[:], x_d[:, c * ks:(c + 1) * ks])
        # part_n[m, j] = sum_k |x[m, 64 j + k]| over this chunk's j's
        nc.vector.tensor_reduce(
            part_n[:, c * jc:(c + 1) * jc],
            x_c[:].rearrange("p (j k) -> p j k", k=BLOCK_K),
            axis=AX.X, op=ALU.add, apply_absolute_value=True)

    # consts on the scalar ring (after the transpose dispatch)
    e_sb = const.tile([128, NBM], F32)
    nc.scalar.dma_start(e_sb[:], e_d)
    ident = const.tile([128, 128], F16)
    nc.scalar.dma_start(ident[:], id_d)
    jh = const.tile([64, 128], F16)
    nc.scalar.dma_start(jh[:], jh_d)
    ksel = const.tile([64, NKT], F16)
    nc.scalar.dma_start(ksel[:], ksel_d)
    bias_sb = const.tile([1, NLOC], F16)
    nc.scalar.dma_start(bias_sb[:], b_d)

    # ba_ps[b, j] = sum_m E[m, b] * part_n[m, j]  (block sums, b on partitions)
    ba_ps = psum.tile([NBM, NBK], F32, tag="ba_ps")
    nc.tensor.matmul(ba_ps[:], lhsT=e_sb[:], rhs=part_n[:], start=True, stop=True)

    # mean = sum / 2048 (exact power of two), rounded to f16 like jnp.mean
    ba16 = mk.tile([NBM, NBK], F16)
    nc.vector.tensor_scalar_mul(ba16[:], ba_ps[:], 1.0 / 2048.0)

    # arow[i, b*64+j] = a[b, j] on 64 partitions, via block-diag expand + matmul
    # rhs3[c, b*64+j] = a[c, j] * [c == b]
    rhs3 = mk.tile([NBM, NBM * NBK], F16)
    nc.vector.tensor_tensor(
        rhs3[:].rearrange("c (b j) -> c b j", b=NBM),
        ba16[:].unsqueeze(1).broadcast_to((NBM, NBM, NBK)),
        ident[0:NBM, 0:NBM].unsqueeze(-1).broadcast_to((NBM, NBM, NBK)),
        op=ALU.mult)
    ones4c = mk.tile([NBM, 64], F16)
    nc.vector.memset(ones4c[:], 1.0)
    arow_ps = psum.tile([64, NBM * NBK], F32, tag="arow_ps")
    nc.tensor.matmul(arow_ps[:], lhsT=ones4c[:], rhs=rhs3[:], start=True, stop=True)
    arow = mk.tile([64, NBM * NBK], F16)
    nc.vector.tensor_copy(arow[:], arow_ps[:])

    # acol[i, b] = a[b, i] via PE transpose
    acol_ps = psum.tile([64, NBM], F16, tag="acol_ps")
    nc.tensor.transpose(acol_ps[:], ba16[:], ident[0:NBM, 0:NBM])
    acol = mk.tile([64, NBM], F16)
    nc.vector.tensor_copy(acol[:], acol_ps[:])

    # cnt[i, b] = #{j : a[b, j] > a[b, i]};  keep iff cnt < KEEP
    cmp = mk.tile([64, NBM * NBK], F16)
    nc.vector.tensor_tensor(
        cmp[:].rearrange("i (b j) -> i b j", b=NBM),
        arow[:].rearrange("i (b j) -> i b j", b=NBM),
        acol[:].unsqueeze(-1).broadcast_to((64, NBM, NBK)),
        op=ALU.is_gt)
    cnt = mk.tile([64, NBM], F32)
    nc.vector.tensor_reduce(cnt[:], cmp[:].rearrange("i (b j) -> i b j", b=NBM),
                            axis=AX.X, op=ALU.add)
    keep16 = mk.tile([64, NBM], F16)
    nc.vector.tensor_scalar(keep16[:], cnt[:], float(KEEP), None, op0=ALU.is_lt)

    # keep_scal[p, b*32+kt] = keep16[2kt + p//64, b]
    #   = sum_j [j%2 == p//64] * keep16[j, b] * [j//2 == kt]  (factored selector)
    # rhs2[j, b*32+kt] = keep16[j, b] * Ksel[j, kt]
    rhs2 = mk.tile([64, 128], F16)
    nc.vector.tensor_tensor(
        rhs2[:].rearrange("j (b kt) -> j b kt", b=NBM),
        keep16[:].unsqueeze(-1).broadcast_to((64, NBM, NKT)),
        ksel[:].unsqueeze(1).broadcast_to((64, NBM, NKT)),
        op=ALU.mult)
    ks_ps = psum.tile([128, 128], F32, tag="ks_ps")
    nc.tensor.matmul(ks_ps[:], lhsT=jh[:], rhs=rhs2[:], start=True, stop=True)
    keep_scal = mk.tile([128, 128], F16)
    nc.vector.tensor_copy(keep_scal[:], ks_ps[:])

    ones = const.tile([1, 128], F16)
    nc.vector.memset(ones[:], 1.0)

    # ---- main GEMM: out[m, n] = sum_kt xm_kt.T @ w_kt + ones.T @ bias ----
    xmpool = ctx.enter_context(tc.tile_pool(name="xmpool", bufs=NKT))
    pbanks = [psum.tile([128, 512], F32, name=f"pn{i}", tag=f"pn{i}")
              for i in range(3)]
    w_engines = [nc.sync, nc.gpsimd, nc.scalar]
    for kt in range(NKT):
        w_t = wpool.tile([128, NLOC], F16)
        w_engines[kt % 3].dma_start(w_t[:], w_d[kt * 128:(kt + 1) * 128, :])
        # masked xT for this k tile: xm[p, b, m] = xt * keep[2kt+p//64, b]
        xm_t = xmpool.tile([128, 128], F16, name=f"xm{kt}", tag="xm")
        nc.vector.tensor_tensor(
            xm_t[:].rearrange("p (b m) -> p b m", b=NBM),
            xt[:, kt * 128:(kt + 1) * 128].rearrange("p (b m) -> p b m", b=NBM),
            keep_scal[:, kt:kt + 97:32].unsqueeze(-1).broadcast_to((128, NBM, BLOCK_M)),
            op=ALU.mult)
        for nt, (n0, nsz) in enumerate(N_TILES):
            nc.tensor.matmul(pbanks[nt][:, :nsz],
                             lhsT=xm_t[:],
                             rhs=w_t[:, n0:n0 + nsz],
                             start=(kt == 0), stop=False)
    out_sb = opool.tile([128, NLOC], F16)
    for nt, (n0, nsz) in enumerate(N_TILES):
        nc.tensor.matmul(pbanks[nt][:, :nsz], lhsT=ones[:],
                         rhs=bias_sb[:, n0:n0 + nsz], start=False, stop=True)
        nc.scalar.activation(out_sb[:, n0:n0 + nsz], pbanks[nt][:, :nsz], ACT.Copy)
        nc.scalar.dma_start(o_d[:, n0:n0 + nsz], out_sb[:, n0:n0 + nsz])


_CACHE = {}


def _build():
    if "nc" in _CACHE:
        return _CACHE["nc"]
    nc = bacc.Bacc("TRN2", target_bir_lowering=False, debug=False,
                   num_devices=NCORES)
    x_d = nc.dram_tensor("x", (M, K), F16, kind="ExternalInput").ap()
    w_d = nc.dram_tensor("w", (K, NLOC), F16, kind="ExternalInput").ap()
    b_d = nc.dram_tensor("bias", (1, NLOC), F16, kind="ExternalInput").ap()
    e_d = nc.dram_tensor("E", (M, NBM), F32, kind="ExternalInput").ap()
    id_d = nc.dram_tensor("ident", (128, 128), F16, kind="ExternalInput").ap()
    jh_d = nc.dram_tensor("JH", (64, 128), F16, kind="ExternalInput").ap()
    ksel_d = nc.dram_tensor("Ksel", (64, NKT), F16, kind="ExternalInput").ap()
    o_d = nc.dram_tensor("out", (M, NLOC), F16, kind="ExternalOutput").ap()
    with tile.TileContext(nc) as tc:
        with ExitStack() as ctx:
            _program(ctx, tc, [x_d, w_d, b_d, e_d, id_d, jh_d, ksel_d], [o_d])
    nc.compile()
    _CACHE["nc"] = nc
    return nc


def _make_in_maps(x2, weight, bias):
    e_np = np.zeros((M, NBM), np.float32)
    for b in range(NBM):
        e_np[b * BLOCK_M:(b + 1) * BLOCK_M, b] = 1.0
    id_np = np.eye(128, dtype=np.float16)
    j_idx = np.arange(64)
    jh_np = (j_idx[:, None] % 2 == (np.arange(128)[None, :] // 64)).astype(np.float16)
    ksel_np = (j_idx[:, None] // 2 == np.arange(NKT)[None, :]).astype(np.float16)

    in_maps = []
    for c in range(NCORES):
        sl = slice(c * NLOC, (c + 1) * NLOC)
        in_maps.append({
            "x": x2,
            "w": np.ascontiguousarray(weight[:, sl].astype(np.float16, copy=False)),
            "bias": np.ascontiguousarray(
                np.asarray(bias)[sl].astype(np.float16, copy=False).reshape(1, NLOC)),
            "E": e_np,
            "ident": id_np,
            "JH": jh_np,
            "Ksel": ksel_np,
        })
    return in_maps


def kernel(x: np.ndarray, weight: np.ndarray, bias: np.ndarray) -> np.ndarray:
    x = np.asarray(x)
    weight = np.asarray(weight)
    bias = np.asarray(bias)
    bsz, seq, hidden = x.shape
    assert (bsz, seq, hidden) == (M, 1, K) and weight.shape == (K, N)

    x2 = np.ascontiguousarray(x.reshape(M, K).astype(np.float16, copy=False))
    in_maps = _make_in_maps(x2, weight, bias)
    nc = _build()
    res = run_bass_kernel_spmd(nc, in_maps, core_ids=list(range(NCORES)))
    out = np.concatenate([r["out"] for r in res.results], axis=1)
    return out.reshape(M, 1, N).astype(x.dtype, copy=False)


if __name__ == "__main__":
    rng = np.random.default_rng(0)
    x = rng.standard_normal((M, 1, K)).astype(np.float16)
    w = (rng.standard_normal((K, N)) * 0.01).astype(np.float16)
    b = np.zeros((N,), np.float16)
    out = kernel(x, w, b)
    print(out.shape, out.dtype)


# revision 17
# speedup vs baseline: 1.0447x; 1.0447x over previous
"""Block-sparse top-k masked linear for Trainium2, tensor-parallel over 8 cores.

out = (block_masked x) @ W + bias
  x: (128, 1, 4096) fp16, W: (4096, 11008) fp16, bias: (11008,) fp16
  mask: per (32-row x 64-col) block of x, keep blocks whose mean |x| is
  >= the 32nd-largest of the 64 k-block activations in that row block.

Sharding: column-parallel — each of the 8 cores gets an 11008/8 = 1376
column slice of W and bias; x is replicated; outputs are concatenated.
"""
from contextlib import ExitStack

import numpy as np

import concourse.bass as bass
import concourse.tile as tile
from concourse import bacc, mybir
from concourse.bass_utils import run_bass_kernel_spmd

F16 = mybir.dt.float16
F32 = mybir.dt.float32
AX = mybir.AxisListType
ALU = mybir.AluOpType
ACT = mybir.ActivationFunctionType

M = 128          # rows of x
K = 4096         # contraction
N = 11008        # out features
NCORES = 8
NLOC = N // NCORES           # 1376 columns per core
BLOCK_M, BLOCK_K = 32, 64
NBM, NBK = M // BLOCK_M, K // BLOCK_K   # 4 row blocks, 64 k blocks
KEEP = 32                               # k blocks kept per row block
NKT = K // 128                          # 32 k tiles of 128
N_TILES = [(0, 512), (512, 512), (1024, 352)]   # n-tile offsets/sizes
W_BUFS = 12


def _program(ctx: ExitStack, tc: tile.TileContext, ins, outs):
    nc = tc.nc
    x_d, xtr_d, w_d, b_d, e_d, id_d, jh_d, ksel_d = ins
    (o_d,) = outs

    const = ctx.enter_context(tc.tile_pool(name="const", bufs=1))
    xbuf = ctx.enter_context(tc.tile_pool(name="xbuf", bufs=1))
    mk = ctx.enter_context(tc.tile_pool(name="mk", bufs=1))
    wpool = ctx.enter_context(tc.tile_pool(name="wpool", bufs=W_BUFS))
    opool = ctx.enter_context(tc.tile_pool(name="opool", bufs=1))
    psum = ctx.enter_context(tc.tile_pool(name="psum", bufs=1, space="PSUM"))

    # ---- x transposed for the GEMM: xt[p, kt*128 + m] = x[m, 128 kt + p] ----
    # one xbar-transpose call on the scalar ring: out[p, kt, m] = in[m, kt*128+p]
    xt = xbuf.tile([128, K], F16)
    nc.scalar.dma_start_transpose(
        xt[:].rearrange("p (kt m) -> p kt m", kt=NKT), xtr_d)

    # ---- mask pipeline (natural x layout; independent of the transpose) ----
    # x chunks in separate tiles so loads don't WAR-serialize behind reduces
    NCH = 8
    xc = ctx.enter_context(tc.tile_pool(name="xc", bufs=NCH))
    part_n = mk.tile([128, NBK], F32)
    jc = NBK // NCH
    ks = K // NCH
    for c in range(NCH):
        x_c = xc.tile([128, ks], F16, name=f"xch{c}", tag="xch")
        nc.sync.dma_start(x_c[:], x_d[:, c * ks:(c + 1) * ks])
        # part_n[m, j] = sum_k |x[m, 64 j + k]| over this chunk's j's
        nc.vector.tensor_reduce(
            part_n[:, c * jc:(c + 1) * jc],
            x_c[:].rearrange("p (j k) -> p j k", k=BLOCK_K),
            axis=AX.X, op=ALU.add, apply_absolute_value=True)

    # consts on the scalar ring (after the transpose dispatch)
    e_sb = const.tile([128, NBM], F32)
    nc.scalar.dma_start(e_sb[:], e_d)
    ident = const.tile([128, 128], F16)
    nc.scalar.dma_start(ident[:], id_d)
    jh = const.tile([64, 128], F16)
    nc.scalar.dma_start(jh[:], jh_d)
    ksel = const.tile([64, NKT], F16)
    nc.scalar.dma_start(ksel[:], ksel_d)
    bias_sb = const.tile([1, NLOC], F16)
    nc.scalar.dma_start(bias_sb[:], b_d)

    # ba_ps[b, j] = sum_m E[m, b] * part_n[m, j]  (block sums, b on partitions)
    ba_ps = psum.tile([NBM, NBK], F32, tag="ba_ps")
    nc.tensor.matmul(ba_ps[:], lhsT=e_sb[:], rhs=part_n[:], start=True, stop=True)

    # mean = sum / 2048 (exact power of two), rounded to f16 like jnp.mean
    ba16 = mk.tile([NBM, NBK], F16)
    nc.vector.tensor_scalar_mul(ba16[:], ba_ps[:], 1.0 / 2048.0)

    # arow[i, b*64+j] = a[b, j] on 64 partitions, via block-diag expand + matmul
    # rhs3[c, b*64+j] = a[c, j] * [c == b]
    rhs3 = mk.tile([NBM, NBM * NBK], F16)
    nc.vector.tensor_tensor(
        rhs3[:].rearrange("c (b j) -> c b j", b=NBM),
        ba16[:].unsqueeze(1).broadcast_to((NBM, NBM, NBK)),
        ident[0:NBM, 0:NBM].unsqueeze(-1).broadcast_to((NBM, NBM, NBK)),
        op=ALU.mult)
    ones4c = mk.tile([NBM, 64], F16)
    nc.vector.memset(ones4c[:], 1.0)
    arow_ps = psum.tile([64, NBM * NBK], F32, tag="arow_ps")
    nc.tensor.matmul(arow_ps[:], lhsT=ones4c[:], rhs=rhs3[:], start=True, stop=True)
    arow = mk.tile([64, NBM * NBK], F16)
    nc.vector.tensor_copy(arow[:], arow_ps[:])

    # acol[i, b] = a[b, i] via PE transpose
    acol_ps = psum.tile([64, NBM], F16, tag="acol_ps")
    nc.tensor.transpose(acol_ps[:], ba16[:], ident[0:NBM, 0:NBM])
    acol = mk.tile([64, NBM], F16)
    nc.vector.tensor_copy(acol[:], acol_ps[:])

    # cnt[i, b] = #{j : a[b, j] > a[b, i]};  keep iff cnt < KEEP
    cmp = mk.tile([64, NBM * NBK], F16)
    nc.vector.tensor_tensor(
        cmp[:].rearrange("i (b j) -> i b j", b=NBM),
        arow[:].rearrange("i (b j) -> i b j", b=NBM),
        acol[:].unsqueeze(-1).broadcast_to((64, NBM, NBK)),
        op=ALU.is_gt)
    cnt = mk.tile([64, NBM], F32)
    nc.vector.tensor_reduce(cnt[:], cmp[:].rearrange("i (b j) -> i b j", b=NBM),
                            axis=AX.X, op=ALU.add)
    keep16 = mk.tile([64, NBM], F16)
    nc.vector.tensor_scalar(keep16[:], cnt[:], float(KEEP), None, op0=ALU.is_lt)

    # keep_scal[p, b*32+kt] = keep16[2kt + p//64, b]
    #   = sum_j [j%2 == p//64] * keep16[j, b] * [j//2 == kt]  (factored selector)
    # rhs2[j, b*32+kt] = keep16[j, b] * Ksel[j, kt]
    rhs2 = mk.tile([64, 128], F16)
    nc.vector.tensor_tensor(
        rhs2[:].rearrange("j (b kt) -> j b kt", b=NBM),
        keep16[:].unsqueeze(-1).broadcast_to((64, NBM, NKT)),
        ksel[:].unsqueeze(1).broadcast_to((64, NBM, NKT)),
        op=ALU.mult)
    ks_ps = psum.tile([128, 128], F32, tag="ks_ps")
    nc.tensor.matmul(ks_ps[:], lhsT=jh[:], rhs=rhs2[:], start=True, stop=True)
    keep_scal = mk.tile([128, 128], F16)
    nc.vector.tensor_copy(keep_scal[:], ks_ps[:])

    ones = const.tile([1, 128], F16)
    nc.vector.memset(ones[:], 1.0)

    # ---- main GEMM: out[m, n] = sum_kt xm_kt.T @ w_kt + ones.T @ bias ----
    xmpool = ctx.enter_context(tc.tile_pool(name="xmpool", bufs=NKT))
    pbanks = [psum.tile([128, 512], F32, name=f"pn{i}", tag=f"pn{i}")
              for i in range(3)]
    w_engines = [nc.sync, nc.gpsimd, nc.scalar]
    for kt in range(NKT):
        w_t = wpool.tile([128, NLOC], F16)
        w_engines[kt % 3].dma_start(w_t[:], w_d[kt * 128:(kt + 1) * 128, :])
        # masked xT for this k tile: xm[p, b, m] = xt * keep[2kt+p//64, b]
        xm_t = xmpool.tile([128, 128], F16, name=f"xm{kt}", tag="xm")
        nc.vector.tensor_tensor(
            xm_t[:].rearrange("p (b m) -> p b m", b=NBM),
            xt[:, kt * 128:(kt + 1) * 128].rearrange("p (b m) -> p b m", b=NBM),
            keep_scal[:, kt:kt + 97:32].unsqueeze(-1).broadcast_to((128, NBM, BLOCK_M)),
            op=ALU.mult)
        for nt, (n0, nsz) in enumerate(N_TILES):
            nc.tensor.matmul(pbanks[nt][:, :nsz],
                             lhsT=xm_t[:],
                             rhs=w_t[:, n0:n0 + nsz],
                             start=(kt == 0), stop=False)
    out_sb = opool.tile([128, NLOC], F16)
    for nt, (n0, nsz) in enumerate(N_TILES):
        nc.tensor.matmul(pbanks[nt][:, :nsz], lhsT=ones[:],
                         rhs=bias_sb[:, n0:n0 + nsz], start=False, stop=True)
        nc.scalar.activation(out_sb[:, n0:n0 + nsz], pbanks[nt][:, :nsz], ACT.Copy)
        nc.scalar.dma_start(o_d[:, n0:n0 + nsz], out_sb[:, n0:n0 + nsz])


_CACHE = {}


def _build():
    if "nc" in _CACHE:
        return _CACHE["nc"]
    nc = bacc.Bacc("TRN2", target_bir_lowering=False, debug=False,
                   num_devices=NCORES)
    x_d = nc.dram_tensor("x", (M, K), F16, kind="ExternalInput").ap()
    xtr_d = nc.dram_tensor("xtr", (M, K), F16, kind="ExternalInput").ap()
    w_d = nc.dram_tensor("w", (K, NLOC), F16, kind="ExternalInput").ap()
    b_d = nc.dram_tensor("bias", (1, NLOC), F16, kind="ExternalInput").ap()
    e_d = nc.dram_tensor("E", (M, NBM), F32, kind="ExternalInput").ap()
    id_d = nc.dram_tensor("ident", (128, 128), F16, kind="ExternalInput").ap()
    jh_d = nc.dram_tensor("JH", (64, 128), F16, kind="ExternalInput").ap()
    ksel_d = nc.dram_tensor("Ksel", (64, NKT), F16, kind="ExternalInput").ap()
    o_d = nc.dram_tensor("out", (M, NLOC), F16, kind="ExternalOutput").ap()
    with tile.TileContext(nc) as tc:
        with ExitStack() as ctx:
            _program(ctx, tc, [x_d, xtr_d, w_d, b_d, e_d, id_d, jh_d, ksel_d], [o_d])
    nc.compile()
    _CACHE["nc"] = nc
    return nc


def _make_in_maps(x2, weight, bias):
    e_np = np.zeros((M, NBM), np.float32)
    for b in range(NBM):
        e_np[b * BLOCK_M:(b + 1) * BLOCK_M, b] = 1.0
    id_np = np.eye(128, dtype=np.float16)
    j_idx = np.arange(64)
    jh_np = (j_idx[:, None] % 2 == (np.arange(128)[None, :] // 64)).astype(np.float16)
    ksel_np = (j_idx[:, None] // 2 == np.arange(NKT)[None, :]).astype(np.float16)

    in_maps = []
    for c in range(NCORES):
        sl = slice(c * NLOC, (c + 1) * NLOC)
        in_maps.append({
            "x": x2,
            "xtr": x2,
            "w": np.ascontiguousarray(weight[:, sl].astype(np.float16, copy=False)),
            "bias": np.ascontiguousarray(
                np.asarray(bias)[sl].astype(np.float16, copy=False).reshape(1, NLOC)),
            "E": e_np,
            "ident": id_np,
            "JH": jh_np,
            "Ksel": ksel_np,
        })
    return in_maps


def kernel(x: np.ndarray, weight: np.ndarray, bias: np.ndarray) -> np.ndarray:
    x = np.asarray(x)
    weight = np.asarray(weight)
    bias = np.asarray(bias)
    bsz, seq, hidden = x.shape
    assert (bsz, seq, hidden) == (M, 1, K) and weight.shape == (K, N)

    x2 = np.ascontiguousarray(x.reshape(M, K).astype(np.float16, copy=False))
    in_maps = _make_in_maps(x2, weight, bias)
    nc = _build()
    res = run_bass_kernel_spmd(nc, in_maps, core_ids=list(range(NCORES)))
    out = np.concatenate([r["out"] for r in res.results], axis=1)
    return out.reshape(M, 1, N).astype(x.dtype, copy=False)


if __name__ == "__main__":
    rng = np.random.default_rng(0)
    x = rng.standard_normal((M, 1, K)).astype(np.float16)
    w = (rng.standard_normal((K, N)) * 0.01).astype(np.float16)
    b = np.zeros((N,), np.float16)
    out = kernel(x, w, b)
    print(out.shape, out.dtype)


# revision 18
# speedup vs baseline: 1.0576x; 1.0124x over previous
"""Block-sparse top-k masked linear for Trainium2, tensor-parallel over 8 cores.

out = (block_masked x) @ W + bias
  x: (128, 1, 4096) fp16, W: (4096, 11008) fp16, bias: (11008,) fp16
  mask: per (32-row x 64-col) block of x, keep blocks whose mean |x| is
  >= the 32nd-largest of the 64 k-block activations in that row block.

Sharding: column-parallel — each of the 8 cores gets an 11008/8 = 1376
column slice of W and bias; x is replicated; outputs are concatenated.
"""
from contextlib import ExitStack

import numpy as np

import concourse.bass as bass
import concourse.tile as tile
from concourse import bacc, mybir
from concourse.bass_utils import run_bass_kernel_spmd

F16 = mybir.dt.float16
F32 = mybir.dt.float32
AX = mybir.AxisListType
ALU = mybir.AluOpType
ACT = mybir.ActivationFunctionType

M = 128          # rows of x
K = 4096         # contraction
N = 11008        # out features
NCORES = 8
NLOC = N // NCORES           # 1376 columns per core
BLOCK_M, BLOCK_K = 32, 64
NBM, NBK = M // BLOCK_M, K // BLOCK_K   # 4 row blocks, 64 k blocks
KEEP = 32                               # k blocks kept per row block
NKT = K // 128                          # 32 k tiles of 128
N_TILES = [(0, 512), (512, 512), (1024, 352)]   # n-tile offsets/sizes
W_BUFS = 12


def _program(ctx: ExitStack, tc: tile.TileContext, ins, outs):
    nc = tc.nc
    x_d, xtr_d, w_d, b_d, e_d, id_d, jh_d, ksel_d = ins
    (o_d,) = outs

    const = ctx.enter_context(tc.tile_pool(name="const", bufs=1))
    xbuf = ctx.enter_context(tc.tile_pool(name="xbuf", bufs=1))
    mk = ctx.enter_context(tc.tile_pool(name="mk", bufs=1))
    wpool = ctx.enter_context(tc.tile_pool(name="wpool", bufs=W_BUFS))
    opool = ctx.enter_context(tc.tile_pool(name="opool", bufs=1))
    psum = ctx.enter_context(tc.tile_pool(name="psum", bufs=1, space="PSUM"))

    # ---- x transposed for the GEMM: xt[p, kt*128 + m] = x[m, 128 kt + p] ----
    # one xbar-transpose call on the scalar ring: out[p, kt, m] = in[m, kt*128+p]
    xt = xbuf.tile([128, K], F16)
    nc.scalar.dma_start_transpose(
        xt[:].rearrange("p (kt m) -> p kt m", kt=NKT), xtr_d)

    # E early on gpsimd (needed right after the reduces)
    e_sb = const.tile([128, NBM], F32)
    nc.gpsimd.dma_start(e_sb[:], e_d)

    # ---- mask pipeline (natural x layout; independent of the transpose) ----
    # x chunks in separate tiles so loads don't WAR-serialize behind reduces
    NCH = 8
    xc = ctx.enter_context(tc.tile_pool(name="xc", bufs=NCH))
    part_n = mk.tile([128, NBK], F32)
    jc = NBK // NCH
    ks = K // NCH
    for c in range(NCH):
        x_c = xc.tile([128, ks], F16, name=f"xch{c}", tag="xch")
        nc.gpsimd.dma_start(x_c[:], x_d[:, c * ks:(c + 1) * ks])
        # part_n[m, j] = sum_k |x[m, 64 j + k]| over this chunk's j's
        nc.vector.tensor_reduce(
            part_n[:, c * jc:(c + 1) * jc],
            x_c[:].rearrange("p (j k) -> p j k", k=BLOCK_K),
            axis=AX.X, op=ALU.add, apply_absolute_value=True)

    # consts on the gpsimd ring
    ident = const.tile([128, 128], F16)
    nc.gpsimd.dma_start(ident[:], id_d)
    jh = const.tile([64, 128], F16)
    nc.gpsimd.dma_start(jh[:], jh_d)
    ksel = const.tile([64, NKT], F16)
    nc.gpsimd.dma_start(ksel[:], ksel_d)
    bias_sb = const.tile([1, NLOC], F16)
    nc.gpsimd.dma_start(bias_sb[:], b_d)

    # ba_ps[b, j] = sum_m E[m, b] * part_n[m, j]  (block sums, b on partitions)
    ba_ps = psum.tile([NBM, NBK], F32, tag="ba_ps")
    nc.tensor.matmul(ba_ps[:], lhsT=e_sb[:], rhs=part_n[:], start=True, stop=True)

    # mean = sum / 2048 (exact power of two), rounded to f16 like jnp.mean
    ba16 = mk.tile([NBM, NBK], F16)
    nc.vector.tensor_scalar_mul(ba16[:], ba_ps[:], 1.0 / 2048.0)

    # arow[i, b*64+j] = a[b, j] on 64 partitions, via block-diag expand + matmul
    # rhs3[c, b*64+j] = a[c, j] * [c == b]
    rhs3 = mk.tile([NBM, NBM * NBK], F16)
    nc.vector.tensor_tensor(
        rhs3[:].rearrange("c (b j) -> c b j", b=NBM),
        ba16[:].unsqueeze(1).broadcast_to((NBM, NBM, NBK)),
        ident[0:NBM, 0:NBM].unsqueeze(-1).broadcast_to((NBM, NBM, NBK)),
        op=ALU.mult)
    ones4c = mk.tile([NBM, 64], F16)
    nc.vector.memset(ones4c[:], 1.0)
    arow_ps = psum.tile([64, NBM * NBK], F32, tag="arow_ps")
    nc.tensor.matmul(arow_ps[:], lhsT=ones4c[:], rhs=rhs3[:], start=True, stop=True)
    arow = mk.tile([64, NBM * NBK], F16)
    nc.vector.tensor_copy(arow[:], arow_ps[:])

    # acol[i, b] = a[b, i] via PE transpose
    acol_ps = psum.tile([64, NBM], F16, tag="acol_ps")
    nc.tensor.transpose(acol_ps[:], ba16[:], ident[0:NBM, 0:NBM])
    acol = mk.tile([64, NBM], F16)
    nc.vector.tensor_copy(acol[:], acol_ps[:])

    # cnt[i, b] = #{j : a[b, j] > a[b, i]};  keep iff cnt < KEEP
    cmp = mk.tile([64, NBM * NBK], F16)
    nc.vector.tensor_tensor(
        cmp[:].rearrange("i (b j) -> i b j", b=NBM),
        arow[:].rearrange("i (b j) -> i b j", b=NBM),
        acol[:].unsqueeze(-1).broadcast_to((64, NBM, NBK)),
        op=ALU.is_gt)
    cnt = mk.tile([64, NBM], F32)
    nc.vector.tensor_reduce(cnt[:], cmp[:].rearrange("i (b j) -> i b j", b=NBM),
                            axis=AX.X, op=ALU.add)
    keep16 = mk.tile([64, NBM], F16)
    nc.vector.tensor_scalar(keep16[:], cnt[:], float(KEEP), None, op0=ALU.is_lt)

    # keep_scal[p, b*32+kt] = keep16[2kt + p//64, b]
    #   = sum_j [j%2 == p//64] * keep16[j, b] * [j//2 == kt]  (factored selector)
    # rhs2[j, b*32+kt] = keep16[j, b] * Ksel[j, kt]
    rhs2 = mk.tile([64, 128], F16)
    nc.vector.tensor_tensor(
        rhs2[:].rearrange("j (b kt) -> j b kt", b=NBM),
        keep16[:].unsqueeze(-1).broadcast_to((64, NBM, NKT)),
        ksel[:].unsqueeze(1).broadcast_to((64, NBM, NKT)),
        op=ALU.mult)
    ks_ps = psum.tile([128, 128], F32, tag="ks_ps")
    nc.tensor.matmul(ks_ps[:], lhsT=jh[:], rhs=rhs2[:], start=True, stop=True)
    keep_scal = mk.tile([128, 128], F16)
    nc.vector.tensor_copy(keep_scal[:], ks_ps[:])

    ones = const.tile([1, 128], F16)
    nc.vector.memset(ones[:], 1.0)

    # ---- main GEMM: out[m, n] = sum_kt xm_kt.T @ w_kt + ones.T @ bias ----
    xmpool = ctx.enter_context(tc.tile_pool(name="xmpool", bufs=NKT))
    pbanks = [psum.tile([128, 512], F32, name=f"pn{i}", tag=f"pn{i}")
              for i in range(3)]
    w_engines = [nc.sync, nc.gpsimd, nc.scalar]
    for kt in range(NKT):
        w_t = wpool.tile([128, NLOC], F16)
        w_engines[kt % 3].dma_start(w_t[:], w_d[kt * 128:(kt + 1) * 128, :])
        # masked xT for this k tile: xm[p, b, m] = xt * keep[2kt+p//64, b]
        xm_t = xmpool.tile([128, 128], F16, name=f"xm{kt}", tag="xm")
        nc.vector.tensor_tensor(
            xm_t[:].rearrange("p (b m) -> p b m", b=NBM),
            xt[:, kt * 128:(kt + 1) * 128].rearrange("p (b m) -> p b m", b=NBM),
            keep_scal[:, kt:kt + 97:32].unsqueeze(-1).broadcast_to((128, NBM, BLOCK_M)),
            op=ALU.mult)
        for nt, (n0, nsz) in enumerate(N_TILES):
            nc.tensor.matmul(pbanks[nt][:, :nsz],
                             lhsT=xm_t[:],
                             rhs=w_t[:, n0:n0 + nsz],
                             start=(kt == 0), stop=False)
    out_sb = opool.tile([128, NLOC], F16)
    for nt, (n0, nsz) in enumerate(N_TILES):
        nc.tensor.matmul(pbanks[nt][:, :nsz], lhsT=ones[:],
                         rhs=bias_sb[:, n0:n0 + nsz], start=False, stop=True)
        nc.scalar.activation(out_sb[:, n0:n0 + nsz], pbanks[nt][:, :nsz], ACT.Copy)
        nc.scalar.dma_start(o_d[:, n0:n0 + nsz], out_sb[:, n0:n0 + nsz])


_CACHE = {}


def _build():
    if "nc" in _CACHE:
        return _CACHE["nc"]
    nc = bacc.Bacc("TRN2", target_bir_lowering=False, debug=False,
                   num_devices=NCORES)
    x_d = nc.dram_tensor("x", (M, K), F16, kind="ExternalInput").ap()
    xtr_d = nc.dram_tensor("xtr", (M, K), F16, kind="ExternalInput").ap()
    w_d = nc.dram_tensor("w", (K, NLOC), F16, kind="ExternalInput").ap()
    b_d = nc.dram_tensor("bias", (1, NLOC), F16, kind="ExternalInput").ap()
    e_d = nc.dram_tensor("E", (M, NBM), F32, kind="ExternalInput").ap()
    id_d = nc.dram_tensor("ident", (128, 128), F16, kind="ExternalInput").ap()
    jh_d = nc.dram_tensor("JH", (64, 128), F16, kind="ExternalInput").ap()
    ksel_d = nc.dram_tensor("Ksel", (64, NKT), F16, kind="ExternalInput").ap()
    o_d = nc.dram_tensor("out", (M, NLOC), F16, kind="ExternalOutput").ap()
    with tile.TileContext(nc) as tc:
        with ExitStack() as ctx:
            _program(ctx, tc, [x_d, xtr_d, w_d, b_d, e_d, id_d, jh_d, ksel_d], [o_d])
    nc.compile()
    _CACHE["nc"] = nc
    return nc


def _make_in_maps(x2, weight, bias):
    e_np = np.zeros((M, NBM), np.float32)
    for b in range(NBM):
        e_np[b * BLOCK_M:(b + 1) * BLOCK_M, b] = 1.0
    id_np = np.eye(128, dtype=np.float16)
    j_idx = np.arange(64)
    jh_np = (j_idx[:, None] % 2 == (np.arange(128)[None, :] // 64)).astype(np.float16)
    ksel_np = (j_idx[:, None] // 2 == np.arange(NKT)[None, :]).astype(np.float16)

    in_maps = []
    for c in range(NCORES):
        sl = slice(c * NLOC, (c + 1) * NLOC)
        in_maps.append({
            "x": x2,
            "xtr": x2,
            "w": np.ascontiguousarray(weight[:, sl].astype(np.float16, copy=False)),
            "bias": np.ascontiguousarray(
                np.asarray(bias)[sl].astype(np.float16, copy=False).reshape(1, NLOC)),
            "E": e_np,
            "ident": id_np,
            "JH": jh_np,
            "Ksel": ksel_np,
        })
    return in_maps


def kernel(x: np.ndarray, weight: np.ndarray, bias: np.ndarray) -> np.ndarray:
    x = np.asarray(x)
    weight = np.asarray(weight)
    bias = np.asarray(bias)
    bsz, seq, hidden = x.shape
    assert (bsz, seq, hidden) == (M, 1, K) and weight.shape == (K, N)

    x2 = np.ascontiguousarray(x.reshape(M, K).astype(np.float16, copy=False))
    in_maps = _make_in_maps(x2, weight, bias)
    nc = _build()
    res = run_bass_kernel_spmd(nc, in_maps, core_ids=list(range(NCORES)))
    out = np.concatenate([r["out"] for r in res.results], axis=1)
    return out.reshape(M, 1, N).astype(x.dtype, copy=False)


if __name__ == "__main__":
    rng = np.random.default_rng(0)
    x = rng.standard_normal((M, 1, K)).astype(np.float16)
    w = (rng.standard_normal((K, N)) * 0.01).astype(np.float16)
    b = np.zeros((N,), np.float16)
    out = kernel(x, w, b)
    print(out.shape, out.dtype)


# revision 19
# speedup vs baseline: 1.2033x; 1.1378x over previous
"""Block-sparse top-k masked linear for Trainium2, tensor-parallel over 8 cores.

out = (block_masked x) @ W + bias
  x: (128, 1, 4096) fp16, W: (4096, 11008) fp16, bias: (11008,) fp16
  mask: per (32-row x 64-col) block of x, keep blocks whose mean |x| is
  >= the 32nd-largest of the 64 k-block activations in that row block.

Sharding: column-parallel — each of the 8 cores gets an 11008/8 = 1376
column slice of W and bias; x is replicated; outputs are concatenated.
"""
from contextlib import ExitStack

import numpy as np

import concourse.bass as bass
import concourse.tile as tile
from concourse import bacc, mybir
from concourse.bass_utils import run_bass_kernel_spmd

F16 = mybir.dt.float16
F32 = mybir.dt.float32
AX = mybir.AxisListType
ALU = mybir.AluOpType
ACT = mybir.ActivationFunctionType

M = 128          # rows of x
K = 4096         # contraction
N = 11008        # out features
NCORES = 8
NLOC = N // NCORES           # 1376 columns per core
BLOCK_M, BLOCK_K = 32, 64
NBM, NBK = M // BLOCK_M, K // BLOCK_K   # 4 row blocks, 64 k blocks
KEEP = 32                               # k blocks kept per row block
NKT = K // 128                          # 32 k tiles of 128
N_TILES = [(0, 512), (512, 512), (1024, 352)]   # n-tile offsets/sizes
W_BUFS = 12


def _program(ctx: ExitStack, tc: tile.TileContext, ins, outs):
    nc = tc.nc
    x_d, xtr_d, w_d, b_d, e_d, id_d, jh_d, ksel_d = ins
    (o_d,) = outs

    const = ctx.enter_context(tc.tile_pool(name="const", bufs=1))
    xbuf = ctx.enter_context(tc.tile_pool(name="xbuf", bufs=1))
    mk = ctx.enter_context(tc.tile_pool(name="mk", bufs=1))
    wpool = ctx.enter_context(tc.tile_pool(name="wpool", bufs=W_BUFS))
    opool = ctx.enter_context(tc.tile_pool(name="opool", bufs=1))
    psum = ctx.enter_context(tc.tile_pool(name="psum", bufs=1, space="PSUM"))

    # consts on the gpsimd ring (ident first: PE transposes need it early)
    ident = const.tile([128, 128], F16)
    nc.gpsimd.dma_start(ident[:], id_d)
    e_sb = const.tile([128, NBM], F32)
    nc.gpsimd.dma_start(e_sb[:], e_d)

    # ---- x chunks on sync (HWDGE): feed both the reduces and PE transposes
    NCH = 8
    TPC = NKT // NCH             # k tiles per chunk
    xc = ctx.enter_context(tc.tile_pool(name="xc", bufs=NCH))
    xtpool = ctx.enter_context(tc.tile_pool(name="xtpool", bufs=NKT))
    part_n = mk.tile([128, NBK], F32)
    jc = NBK // NCH
    ks = K // NCH
    xt_tiles = []
    for c in range(NCH):
        x_c = xc.tile([128, ks], F16, name=f"xch{c}", tag="xch")
        nc.sync.dma_start(x_c[:], x_d[:, c * ks:(c + 1) * ks])
        # part_n[m, j] = sum_k |x[m, 64 j + k]| over this chunk's j's
        nc.vector.tensor_reduce(
            part_n[:, c * jc:(c + 1) * jc],
            x_c[:].rearrange("p (j k) -> p j k", k=BLOCK_K),
            axis=AX.X, op=ALU.add, apply_absolute_value=True)
        # PE-transpose each 128-wide k tile of this chunk: xt[k, m] = x[m, k]
        for t in range(TPC):
            kt = TPC * c + t
            tp = psum.tile([128, 128], F16, name=f"tp{kt}", tag="tp", bufs=3)
            nc.tensor.transpose(tp[:], x_c[:, t * 128:(t + 1) * 128], ident[:])
            xt_t = xtpool.tile([128, 128], F16, name=f"xt{kt}", tag="xt")
            nc.vector.tensor_copy(xt_t[:], tp[:])
            xt_tiles.append(xt_t)

    # remaining consts on the gpsimd ring
    jh = const.tile([64, 128], F16)
    nc.gpsimd.dma_start(jh[:], jh_d)
    ksel = const.tile([64, NKT], F16)
    nc.gpsimd.dma_start(ksel[:], ksel_d)
    bias_sb = const.tile([1, NLOC], F16)
    nc.gpsimd.dma_start(bias_sb[:], b_d)

    # ba_ps[b, j] = sum_m E[m, b] * part_n[m, j]  (block sums, b on partitions)
    ba_ps = psum.tile([NBM, NBK], F32, tag="mkps", bufs=2)
    nc.tensor.matmul(ba_ps[:], lhsT=e_sb[:], rhs=part_n[:], start=True, stop=True)

    # mean = sum / 2048 (exact power of two), rounded to f16 like jnp.mean
    ba16 = mk.tile([NBM, NBK], F16)
    nc.vector.tensor_scalar_mul(ba16[:], ba_ps[:], 1.0 / 2048.0)

    # arow[i, b*64+j] = a[b, j] on 64 partitions, via block-diag expand + matmul
    # rhs3[c, b*64+j] = a[c, j] * [c == b]
    rhs3 = mk.tile([NBM, NBM * NBK], F16)
    nc.vector.tensor_tensor(
        rhs3[:].rearrange("c (b j) -> c b j", b=NBM),
        ba16[:].unsqueeze(1).broadcast_to((NBM, NBM, NBK)),
        ident[0:NBM, 0:NBM].unsqueeze(-1).broadcast_to((NBM, NBM, NBK)),
        op=ALU.mult)
    ones4c = mk.tile([NBM, 64], F16)
    nc.vector.memset(ones4c[:], 1.0)
    arow_ps = psum.tile([64, NBM * NBK], F32, tag="mkps", bufs=2)
    nc.tensor.matmul(arow_ps[:], lhsT=ones4c[:], rhs=rhs3[:], start=True, stop=True)
    arow = mk.tile([64, NBM * NBK], F16)
    nc.vector.tensor_copy(arow[:], arow_ps[:])

    # acol[i, b] = a[b, i] via PE transpose
    acol_ps = psum.tile([64, NBM], F16, tag="mkps", bufs=2)
    nc.tensor.transpose(acol_ps[:], ba16[:], ident[0:NBM, 0:NBM])
    acol = mk.tile([64, NBM], F16)
    nc.vector.tensor_copy(acol[:], acol_ps[:])

    # cnt[i, b] = #{j : a[b, j] > a[b, i]};  keep iff cnt < KEEP
    cmp = mk.tile([64, NBM * NBK], F16)
    nc.vector.tensor_tensor(
        cmp[:].rearrange("i (b j) -> i b j", b=NBM),
        arow[:].rearrange("i (b j) -> i b j", b=NBM),
        acol[:].unsqueeze(-1).broadcast_to((64, NBM, NBK)),
        op=ALU.is_gt)
    cnt = mk.tile([64, NBM], F32)
    nc.vector.tensor_reduce(cnt[:], cmp[:].rearrange("i (b j) -> i b j", b=NBM),
                            axis=AX.X, op=ALU.add)
    keep16 = mk.tile([64, NBM], F16)
    nc.vector.tensor_scalar(keep16[:], cnt[:], float(KEEP), None, op0=ALU.is_lt)

    # keep_scal[p, b*32+kt] = keep16[2kt + p//64, b]
    #   = sum_j [j%2 == p//64] * keep16[j, b] * [j//2 == kt]  (factored selector)
    # rhs2[j, b*32+kt] = keep16[j, b] * Ksel[j, kt]
    rhs2 = mk.tile([64, 128], F16)
    nc.vector.tensor_tensor(
        rhs2[:].rearrange("j (b kt) -> j b kt", b=NBM),
        keep16[:].unsqueeze(-1).broadcast_to((64, NBM, NKT)),
        ksel[:].unsqueeze(1).broadcast_to((64, NBM, NKT)),
        op=ALU.mult)
    ks_ps = psum.tile([128, 128], F32, tag="mkps", bufs=2)
    nc.tensor.matmul(ks_ps[:], lhsT=jh[:], rhs=rhs2[:], start=True, stop=True)
    keep_scal = mk.tile([128, 128], F16)
    nc.vector.tensor_copy(keep_scal[:], ks_ps[:])

    ones = const.tile([1, 128], F16)
    nc.vector.memset(ones[:], 1.0)

    # ---- main GEMM: out[m, n] = sum_kt xm_kt.T @ w_kt + ones.T @ bias ----
    xmpool = ctx.enter_context(tc.tile_pool(name="xmpool", bufs=NKT))
    pbanks = [psum.tile([128, 512], F32, name=f"pn{i}", tag=f"pn{i}")
              for i in range(3)]
    w_engines = [nc.scalar, nc.gpsimd, nc.sync]
    for kt in range(NKT):
        w_t = wpool.tile([128, NLOC], F16)
        w_engines[kt % 3].dma_start(w_t[:], w_d[kt * 128:(kt + 1) * 128, :])
        # masked xT for this k tile: xm[p, b, m] = xt * keep[2kt+p//64, b]
        xm_t = xmpool.tile([128, 128], F16, name=f"xm{kt}", tag="xm")
        nc.vector.tensor_tensor(
            xm_t[:].rearrange("p (b m) -> p b m", b=NBM),
            xt_tiles[kt][:].rearrange("p (b m) -> p b m", b=NBM),
            keep_scal[:, kt:kt + 97:32].unsqueeze(-1).broadcast_to((128, NBM, BLOCK_M)),
            op=ALU.mult)
        for nt, (n0, nsz) in enumerate(N_TILES):
            nc.tensor.matmul(pbanks[nt][:, :nsz],
                             lhsT=xm_t[:],
                             rhs=w_t[:, n0:n0 + nsz],
                             start=(kt == 0), stop=False)
    out_sb = opool.tile([128, NLOC], F16)
    for nt, (n0, nsz) in enumerate(N_TILES):
        nc.tensor.matmul(pbanks[nt][:, :nsz], lhsT=ones[:],
                         rhs=bias_sb[:, n0:n0 + nsz], start=False, stop=True)
        nc.scalar.activation(out_sb[:, n0:n0 + nsz], pbanks[nt][:, :nsz], ACT.Copy)
        nc.scalar.dma_start(o_d[:, n0:n0 + nsz], out_sb[:, n0:n0 + nsz])


_CACHE = {}


def _build():
    if "nc" in _CACHE:
        return _CACHE["nc"]
    nc = bacc.Bacc("TRN2", target_bir_lowering=False, debug=False,
                   num_devices=NCORES)
    x_d = nc.dram_tensor("x", (M, K), F16, kind="ExternalInput").ap()
    xtr_d = nc.dram_tensor("xtr", (M, K), F16, kind="ExternalInput").ap()
    w_d = nc.dram_tensor("w", (K, NLOC), F16, kind="ExternalInput").ap()
    b_d = nc.dram_tensor("bias", (1, NLOC), F16, kind="ExternalInput").ap()
    e_d = nc.dram_tensor("E", (M, NBM), F32, kind="ExternalInput").ap()
    id_d = nc.dram_tensor("ident", (128, 128), F16, kind="ExternalInput").ap()
    jh_d = nc.dram_tensor("JH", (64, 128), F16, kind="ExternalInput").ap()
    ksel_d = nc.dram_tensor("Ksel", (64, NKT), F16, kind="ExternalInput").ap()
    o_d = nc.dram_tensor("out", (M, NLOC), F16, kind="ExternalOutput").ap()
    with tile.TileContext(nc) as tc:
        with ExitStack() as ctx:
            _program(ctx, tc, [x_d, xtr_d, w_d, b_d, e_d, id_d, jh_d, ksel_d], [o_d])
    nc.compile()
    _CACHE["nc"] = nc
    return nc


def _make_in_maps(x2, weight, bias):
    e_np = np.zeros((M, NBM), np.float32)
    for b in range(NBM):
        e_np[b * BLOCK_M:(b + 1) * BLOCK_M, b] = 1.0
    id_np = np.eye(128, dtype=np.float16)
    j_idx = np.arange(64)
    jh_np = (j_idx[:, None] % 2 == (np.arange(128)[None, :] // 64)).astype(np.float16)
    ksel_np = (j_idx[:, None] // 2 == np.arange(NKT)[None, :]).astype(np.float16)

    in_maps = []
    for c in range(NCORES):
        sl = slice(c * NLOC, (c + 1) * NLOC)
        in_maps.append({
            "x": x2,
            "xtr": x2,
            "w": np.ascontiguousarray(weight[:, sl].astype(np.float16, copy=False)),
            "bias": np.ascontiguousarray(
                np.asarray(bias)[sl].astype(np.float16, copy=False).reshape(1, NLOC)),
            "E": e_np,
            "ident": id_np,
            "JH": jh_np,
            "Ksel": ksel_np,
        })
    return in_maps


def kernel(x: np.ndarray, weight: np.ndarray, bias: np.ndarray) -> np.ndarray:
    x = np.asarray(x)
    weight = np.asarray(weight)
    bias = np.asarray(bias)
    bsz, seq, hidden = x.shape
    assert (bsz, seq, hidden) == (M, 1, K) and weight.shape == (K, N)

    x2 = np.ascontiguousarray(x.reshape(M, K).astype(np.float16, copy=False))
    in_maps = _make_in_maps(x2, weight, bias)
    nc = _build()
    res = run_bass_kernel_spmd(nc, in_maps, core_ids=list(range(NCORES)))
    out = np.concatenate([r["out"] for r in res.results], axis=1)
    return out.reshape(M, 1, N).astype(x.dtype, copy=False)


if __name__ == "__main__":
    rng = np.random.default_rng(0)
    x = rng.standard_normal((M, 1, K)).astype(np.float16)
    w = (rng.standard_normal((K, N)) * 0.01).astype(np.float16)
    b = np.zeros((N,), np.float16)
    out = kernel(x, w, b)
    print(out.shape, out.dtype)


# revision 20
# speedup vs baseline: 1.2731x; 1.0580x over previous
"""Block-sparse top-k masked linear for Trainium2, tensor-parallel over 8 cores.

out = (block_masked x) @ W + bias
  x: (128, 1, 4096) fp16, W: (4096, 11008) fp16, bias: (11008,) fp16
  mask: per (32-row x 64-col) block of x, keep blocks whose mean |x| is
  >= the 32nd-largest of the 64 k-block activations in that row block.

Sharding: column-parallel — each of the 8 cores gets an 11008/8 = 1376
column slice of W and bias; x is replicated; outputs are concatenated.
"""
from contextlib import ExitStack

import numpy as np

import concourse.bass as bass
import concourse.tile as tile
from concourse import bacc, mybir
from concourse.bass_utils import run_bass_kernel_spmd

F16 = mybir.dt.float16
F32 = mybir.dt.float32
AX = mybir.AxisListType
ALU = mybir.AluOpType
ACT = mybir.ActivationFunctionType

M = 128          # rows of x
K = 4096         # contraction
N = 11008        # out features
NCORES = 8
NLOC = N // NCORES           # 1376 columns per core
BLOCK_M, BLOCK_K = 32, 64
NBM, NBK = M // BLOCK_M, K // BLOCK_K   # 4 row blocks, 64 k blocks
KEEP = 32                               # k blocks kept per row block
NKT = K // 128                          # 32 k tiles of 128
N_TILES = [(0, 512), (512, 512), (1024, 352)]   # n-tile offsets/sizes
W_BUFS = 16


def _program(ctx: ExitStack, tc: tile.TileContext, ins, outs):
    nc = tc.nc
    x_d, xtr_d, w_d, b_d, e_d, id_d, jh_d, ksel_d = ins
    (o_d,) = outs

    const = ctx.enter_context(tc.tile_pool(name="const", bufs=1))
    xbuf = ctx.enter_context(tc.tile_pool(name="xbuf", bufs=1))
    mk = ctx.enter_context(tc.tile_pool(name="mk", bufs=1))
    wpool = ctx.enter_context(tc.tile_pool(name="wpool", bufs=W_BUFS))
    opool = ctx.enter_context(tc.tile_pool(name="opool", bufs=1))
    psum = ctx.enter_context(tc.tile_pool(name="psum", bufs=1, space="PSUM"))

    # consts on the gpsimd ring (ident first: PE transposes need it early)
    ident = const.tile([128, 128], F16)
    nc.gpsimd.dma_start(ident[:], id_d)
    e_sb = const.tile([128, NBM], F32)
    nc.gpsimd.dma_start(e_sb[:], e_d)

    # ---- HAM warm-up: ~4us of junk matmuls so the PE clock-gate opens
    # before the transposes/GEMM start (otherwise everything runs at 1.2 GHz)
    warm_sb = mk.tile([128, 512], F16)
    nc.vector.memset(warm_sb[:], 0.0)
    warm_ps = psum.tile([128, 512], F32, name="warm_ps", tag="tp", bufs=3)
    for i in range(9):
        nc.tensor.matmul(warm_ps[:], lhsT=warm_sb[:, 0:128], rhs=warm_sb[:],
                         start=True, stop=True)

    # ---- x chunks on sync/scalar (HWDGE): feed the reduces and PE transposes
    NCH = 8
    TPC = NKT // NCH             # k tiles per chunk
    xc = ctx.enter_context(tc.tile_pool(name="xc", bufs=NCH))
    xtpool = ctx.enter_context(tc.tile_pool(name="xtpool", bufs=NKT))
    part_n = mk.tile([128, NBK], F32)
    jc = NBK // NCH
    ks = K // NCH
    xt_tiles = []
    for c in range(NCH):
        x_c = xc.tile([128, ks], F16, name=f"xch{c}", tag="xch")
        (nc.sync if c % 2 == 0 else nc.scalar).dma_start(x_c[:], x_d[:, c * ks:(c + 1) * ks])
        # part_n[m, j] = sum_k |x[m, 64 j + k]| over this chunk's j's
        nc.vector.tensor_reduce(
            part_n[:, c * jc:(c + 1) * jc],
            x_c[:].rearrange("p (j k) -> p j k", k=BLOCK_K),
            axis=AX.X, op=ALU.add, apply_absolute_value=True)
        # PE-transpose each 128-wide k tile of this chunk: xt[k, m] = x[m, k]
        for t in range(TPC):
            kt = TPC * c + t
            tp = psum.tile([128, 128], F16, name=f"tp{kt}", tag="tp", bufs=3)
            nc.tensor.transpose(tp[:], x_c[:, t * 128:(t + 1) * 128], ident[:])
            xt_t = xtpool.tile([128, 128], F16, name=f"xt{kt}", tag="xt")
            nc.vector.tensor_copy(xt_t[:], tp[:])
            xt_tiles.append(xt_t)

    # remaining consts on the gpsimd ring
    jh = const.tile([64, 128], F16)
    nc.gpsimd.dma_start(jh[:], jh_d)
    ksel = const.tile([64, NKT], F16)
    nc.gpsimd.dma_start(ksel[:], ksel_d)
    bias_sb = const.tile([1, NLOC], F16)
    nc.gpsimd.dma_start(bias_sb[:], b_d)

    # ba_ps[b, j] = sum_m E[m, b] * part_n[m, j]  (block sums, b on partitions)
    ba_ps = psum.tile([NBM, NBK], F32, tag="mkps", bufs=2)
    nc.tensor.matmul(ba_ps[:], lhsT=e_sb[:], rhs=part_n[:], start=True, stop=True)

    # mean = sum / 2048 (exact power of two), rounded to f16 like jnp.mean
    ba16 = mk.tile([NBM, NBK], F16)
    nc.vector.tensor_scalar_mul(ba16[:], ba_ps[:], 1.0 / 2048.0)

    # arow[i, b*64+j] = a[b, j] on 64 partitions, via block-diag expand + matmul
    # rhs3[c, b*64+j] = a[c, j] * [c == b]
    rhs3 = mk.tile([NBM, NBM * NBK], F16)
    nc.vector.tensor_tensor(
        rhs3[:].rearrange("c (b j) -> c b j", b=NBM),
        ba16[:].unsqueeze(1).broadcast_to((NBM, NBM, NBK)),
        ident[0:NBM, 0:NBM].unsqueeze(-1).broadcast_to((NBM, NBM, NBK)),
        op=ALU.mult)
    ones4c = mk.tile([NBM, 64], F16)
    nc.vector.memset(ones4c[:], 1.0)
    arow_ps = psum.tile([64, NBM * NBK], F32, tag="mkps", bufs=2)
    nc.tensor.matmul(arow_ps[:], lhsT=ones4c[:], rhs=rhs3[:], start=True, stop=True)
    arow = mk.tile([64, NBM * NBK], F16)
    nc.vector.tensor_copy(arow[:], arow_ps[:])

    # acol[i, b] = a[b, i] via PE transpose
    acol_ps = psum.tile([64, NBM], F16, tag="mkps", bufs=2)
    nc.tensor.transpose(acol_ps[:], ba16[:], ident[0:NBM, 0:NBM])
    acol = mk.tile([64, NBM], F16)
    nc.vector.tensor_copy(acol[:], acol_ps[:])

    # cnt[i, b] = #{j : a[b, j] > a[b, i]};  keep iff cnt < KEEP
    cmp = mk.tile([64, NBM * NBK], F16)
    nc.vector.tensor_tensor(
        cmp[:].rearrange("i (b j) -> i b j", b=NBM),
        arow[:].rearrange("i (b j) -> i b j", b=NBM),
        acol[:].unsqueeze(-1).broadcast_to((64, NBM, NBK)),
        op=ALU.is_gt)
    cnt = mk.tile([64, NBM], F32)
    nc.vector.tensor_reduce(cnt[:], cmp[:].rearrange("i (b j) -> i b j", b=NBM),
                            axis=AX.X, op=ALU.add)
    keep16 = mk.tile([64, NBM], F16)
    nc.vector.tensor_scalar(keep16[:], cnt[:], float(KEEP), None, op0=ALU.is_lt)

    # keep_scal[p, b*32+kt] = keep16[2kt + p//64, b]
    #   = sum_j [j%2 == p//64] * keep16[j, b] * [j//2 == kt]  (factored selector)
    # rhs2[j, b*32+kt] = keep16[j, b] * Ksel[j, kt]
    rhs2 = mk.tile([64, 128], F16)
    nc.vector.tensor_tensor(
        rhs2[:].rearrange("j (b kt) -> j b kt", b=NBM),
        keep16[:].unsqueeze(-1).broadcast_to((64, NBM, NKT)),
        ksel[:].unsqueeze(1).broadcast_to((64, NBM, NKT)),
        op=ALU.mult)
    ks_ps = psum.tile([128, 128], F32, tag="mkps", bufs=2)
    nc.tensor.matmul(ks_ps[:], lhsT=jh[:], rhs=rhs2[:], start=True, stop=True)
    keep_scal = mk.tile([128, 128], F16)
    nc.vector.tensor_copy(keep_scal[:], ks_ps[:])

    ones = const.tile([1, 128], F16)
    nc.vector.memset(ones[:], 1.0)

    # ---- main GEMM: out[m, n] = sum_kt xm_kt.T @ w_kt + ones.T @ bias ----
    xmpool = ctx.enter_context(tc.tile_pool(name="xmpool", bufs=NKT))
    pbanks = [psum.tile([128, 512], F32, name=f"pn{i}", tag=f"pn{i}")
              for i in range(3)]
    w_engines = [nc.scalar, nc.gpsimd, nc.sync]
    for kt in range(NKT):
        w_t = wpool.tile([128, NLOC], F16)
        w_engines[kt % 3].dma_start(w_t[:], w_d[kt * 128:(kt + 1) * 128, :])
        # masked xT for this k tile: xm[p, b, m] = xt * keep[2kt+p//64, b]
        xm_t = xmpool.tile([128, 128], F16, name=f"xm{kt}", tag="xm")
        nc.vector.tensor_tensor(
            xm_t[:].rearrange("p (b m) -> p b m", b=NBM),
            xt_tiles[kt][:].rearrange("p (b m) -> p b m", b=NBM),
            keep_scal[:, kt:kt + 97:32].unsqueeze(-1).broadcast_to((128, NBM, BLOCK_M)),
            op=ALU.mult)
        for nt, (n0, nsz) in enumerate(N_TILES):
            nc.tensor.matmul(pbanks[nt][:, :nsz],
                             lhsT=xm_t[:],
                             rhs=w_t[:, n0:n0 + nsz],
                             start=(kt == 0), stop=False)
    out_sb = opool.tile([128, NLOC], F16)
    for nt, (n0, nsz) in enumerate(N_TILES):
        nc.tensor.matmul(pbanks[nt][:, :nsz], lhsT=ones[:],
                         rhs=bias_sb[:, n0:n0 + nsz], start=False, stop=True)
        nc.scalar.activation(out_sb[:, n0:n0 + nsz], pbanks[nt][:, :nsz], ACT.Copy)
        nc.scalar.dma_start(o_d[:, n0:n0 + nsz], out_sb[:, n0:n0 + nsz])


_CACHE = {}


def _build():
    if "nc" in _CACHE:
        return _CACHE["nc"]
    nc = bacc.Bacc("TRN2", target_bir_lowering=False, debug=False,
                   num_devices=NCORES)
    x_d = nc.dram_tensor("x", (M, K), F16, kind="ExternalInput").ap()
    xtr_d = nc.dram_tensor("xtr", (M, K), F16, kind="ExternalInput").ap()
    w_d = nc.dram_tensor("w", (K, NLOC), F16, kind="ExternalInput").ap()
    b_d = nc.dram_tensor("bias", (1, NLOC), F16, kind="ExternalInput").ap()
    e_d = nc.dram_tensor("E", (M, NBM), F32, kind="ExternalInput").ap()
    id_d = nc.dram_tensor("ident", (128, 128), F16, kind="ExternalInput").ap()
    jh_d = nc.dram_tensor("JH", (64, 128), F16, kind="ExternalInput").ap()
    ksel_d = nc.dram_tensor("Ksel", (64, NKT), F16, kind="ExternalInput").ap()
    o_d = nc.dram_tensor("out", (M, NLOC), F16, kind="ExternalOutput").ap()
    with tile.TileContext(nc) as tc:
        with ExitStack() as ctx:
            _program(ctx, tc, [x_d, xtr_d, w_d, b_d, e_d, id_d, jh_d, ksel_d], [o_d])
    nc.compile()
    _CACHE["nc"] = nc
    return nc


def _make_in_maps(x2, weight, bias):
    e_np = np.zeros((M, NBM), np.float32)
    for b in range(NBM):
        e_np[b * BLOCK_M:(b + 1) * BLOCK_M, b] = 1.0
    id_np = np.eye(128, dtype=np.float16)
    j_idx = np.arange(64)
    jh_np = (j_idx[:, None] % 2 == (np.arange(128)[None, :] // 64)).astype(np.float16)
    ksel_np = (j_idx[:, None] // 2 == np.arange(NKT)[None, :]).astype(np.float16)

    in_maps = []
    for c in range(NCORES):
        sl = slice(c * NLOC, (c + 1) * NLOC)
        in_maps.append({
            "x": x2,
            "xtr": x2,
            "w": np.ascontiguousarray(weight[:, sl].astype(np.float16, copy=False)),
            "bias": np.ascontiguousarray(
                np.asarray(bias)[sl].astype(np.float16, copy=False).reshape(1, NLOC)),
            "E": e_np,
            "ident": id_np,
            "JH": jh_np,
            "Ksel": ksel_np,
        })
    return in_maps


def kernel(x: np.ndarray, weight: np.ndarray, bias: np.ndarray) -> np.ndarray:
    x = np.asarray(x)
    weight = np.asarray(weight)
    bias = np.asarray(bias)
    bsz, seq, hidden = x.shape
    assert (bsz, seq, hidden) == (M, 1, K) and weight.shape == (K, N)

    x2 = np.ascontiguousarray(x.reshape(M, K).astype(np.float16, copy=False))
    in_maps = _make_in_maps(x2, weight, bias)
    nc = _build()
    res = run_bass_kernel_spmd(nc, in_maps, core_ids=list(range(NCORES)))
    out = np.concatenate([r["out"] for r in res.results], axis=1)
    return out.reshape(M, 1, N).astype(x.dtype, copy=False)


if __name__ == "__main__":
    rng = np.random.default_rng(0)
    x = rng.standard_normal((M, 1, K)).astype(np.float16)
    w = (rng.standard_normal((K, N)) * 0.01).astype(np.float16)
    b = np.zeros((N,), np.float16)
    out = kernel(x, w, b)
    print(out.shape, out.dtype)
